# revision 23
# baseline (speedup 1.0000x reference)
"""Trainium2 Bass kernel for nn_EncoderLayer (B=2, T=2048, D=1024, H=16, DFF=4096).

Sharding: 8 cores; core c handles batch b=c//4, query rows q0=(c%4)*512..+512.
Each core redundantly computes LN1 + K,V for its full batch (no collectives),
then attention + output proj + FFN for its own 512 query rows.
Outputs per core: attn slice (16, 512, 2048) and x_out slice (512, 1024).

Numerics: layer-norm affine params are folded into the adjacent projection
weights on the host (exact). Projections/scores/PV/FFN matmuls run in bf16
with fp32 PSUM accumulation; softmax (mask, exp, normalize) runs in fp32.
"""

import sys

sys.path.insert(0, "/opt/trn_rl_repo")

import numpy as np

import concourse.bacc as bacc
import concourse.bass as bass
import concourse.tile as tile
from concourse import mybir
from concourse import bass_utils
from concourse.masks import make_identity

P = 128
B, T, D = 2, 2048, 1024
H, HD = 16, 64
DFF = 4096
TQ = 512          # query rows per core
NQT = TQ // P     # 4
NKT = T // P      # 16
NDT = D // P      # 8
NFT = DFF // P    # 32
EPS = 1e-5
NEG = -1e9        # exp(x + NEG) == 0.0 in fp32 for any realistic score x

f32 = mybir.dt.float32
f32r = mybir.dt.float32r
bf16 = mybir.dt.bfloat16
u8 = mybir.dt.uint8
AF = mybir.ActivationFunctionType
OP = mybir.AluOpType

_CACHE = {}


def _layer_norm_inplace(nc, stats, x_i, eps1):
    """x_i <- (x_i - mean) * rsqrt(var + eps) (ln affine folded into weights)."""
    st = stats.tile([P, 2, 6], f32, tag="bn")
    nc.vector.bn_stats(out=st[:, 0, :], in_=x_i[:, 0:512])
    nc.vector.bn_stats(out=st[:, 1, :], in_=x_i[:, 512:1024])
    mv = stats.tile([P, 2], f32, tag="mv")
    nc.vector.bn_aggr(out=mv, in_=st)
    mean = mv[:, 0:1]
    var = mv[:, 1:2]
    std = stats.tile([P, 1], f32, tag="std")
    nc.scalar.activation(out=std, in_=var, func=AF.Sqrt, bias=eps1, scale=1.0)
    rstd = stats.tile([P, 1], f32, tag="rstd")
    nc.vector.reciprocal(out=rstd, in_=std)
    negmur = stats.tile([P, 1], f32, tag="negmur")
    nc.vector.tensor_scalar(
        out=negmur, in0=mean, scalar1=rstd, scalar2=-1.0, op0=OP.mult, op1=OP.mult
    )
    nc.scalar.activation(out=x_i, in_=x_i, func=AF.Identity, bias=negmur, scale=rstd)


def build(nc):
    # ---------------- DRAM I/O ----------------
    x_batch = nc.dram_tensor("x_batch", (T, D), f32, kind="ExternalInput").ap()
    x_own = nc.dram_tensor("x_own", (TQ, D), f32, kind="ExternalInput").ap()
    mask_q = nc.dram_tensor("mask_q", (H, TQ, T), u8, kind="ExternalInput").ap()
    Wq = nc.dram_tensor("Wq", (D, D), bf16, kind="ExternalInput").ap()
    Wk = nc.dram_tensor("Wk", (D, D), bf16, kind="ExternalInput").ap()
    Wv = nc.dram_tensor("Wv", (D, D), bf16, kind="ExternalInput").ap()
    Wo = nc.dram_tensor("Wo", (D, D), bf16, kind="ExternalInput").ap()
    W1 = nc.dram_tensor("W1", (D, DFF), bf16, kind="ExternalInput").ap()
    W2 = nc.dram_tensor("W2", (DFF, D), bf16, kind="ExternalInput").ap()
    bq = nc.dram_tensor("bq", (D,), f32, kind="ExternalInput").ap()
    bk = nc.dram_tensor("bk", (D,), f32, kind="ExternalInput").ap()
    bv = nc.dram_tensor("bv", (D,), f32r, kind="ExternalInput").ap()
    bo = nc.dram_tensor("bo", (D,), f32, kind="ExternalInput").ap()
    b1 = nc.dram_tensor("b1", (DFF,), f32, kind="ExternalInput").ap()
    b2 = nc.dram_tensor("b2", (D,), f32r, kind="ExternalInput").ap()
    attn_out = nc.dram_tensor("attn_out", (H, TQ, T), f32, kind="ExternalOutput").ap()
    x_out = nc.dram_tensor("x_out", (TQ, D), f32, kind="ExternalOutput").ap()
    ctx_dram = nc.dram_tensor("ctx_scratch", (P, NDT, TQ), bf16).ap()
    r_dram = nc.dram_tensor("r_scratch", (H // 2, 2, NQT, P), f32).ap()

    def bcast(pool, src, n, tag):
        t = pool.tile([P, n], f32, tag=tag, name=tag)
        src_b = bass.AP(tensor=src.tensor, offset=src.offset, ap=[[0, P]] + src.ap)
        nc.sync.dma_start(out=t, in_=src_b)
        return t

    with tile.TileContext(nc) as tc:
        with tc.tile_pool(name="consts", bufs=1) as consts, \
             tc.tile_pool(name="stats", bufs=8) as stats, \
             tc.tile_pool(name="hpool", bufs=5) as hpool:
            ident_bf = consts.tile([P, P], bf16)
            make_identity(nc, ident_bf)
            ident_f = consts.tile([P, P], f32)
            make_identity(nc, ident_f)
            eps1 = consts.tile([P, 1], f32)
            nc.vector.memset(eps1, EPS)
            ones_f = consts.tile([1, P], f32)
            nc.vector.memset(ones_f, 1.0)
            ones_row = consts.tile([1, P], f32r)
            nc.scalar.copy(out=ones_row, in_=ones_f)
            bkT = consts.tile([P, NDT], f32)
            nc.sync.dma_start(out=bkT, in_=bk.rearrange("(f p) -> p f", p=P))
            bqT = consts.tile([P, NDT], f32)
            nc.sync.dma_start(out=bqT, in_=bq.rearrange("(f p) -> p f", p=P))
            b1T = consts.tile([P, NFT], f32)
            nc.sync.dma_start(out=b1T, in_=b1.rearrange("(f p) -> p f", p=P))
            bv_row = consts.tile([1, D], f32r)
            nc.sync.dma_start(out=bv_row, in_=bv[None, :])
            b2_row = consts.tile([1, D], f32r)
            nc.sync.dma_start(out=b2_row, in_=b2[None, :])

            # ---------- Stages A-C under kv_pool ----------
            kv_ctx = tc.tile_pool(name="kv_pool", bufs=1)
            kv_pool = kv_ctx.__enter__()
            ctxT = kv_pool.tile([P, NDT, TQ], bf16)   # ctx^T [feat, qtok]  8KB/part
            KT = kv_pool.tile([P, NDT, T], bf16)      # K^T  [feat, tok]   32KB/part
            Vb = kv_pool.tile([P, NKT, D], bf16)      # V    [tok, feat]   32KB/part
            QT = kv_pool.tile([P, NDT, TQ], bf16)     # Q^T/8 [feat, qtok]  8KB/part

            # ---------- Stage A: LN1 -> hT ----------
            hT_ctx = tc.tile_pool(name="hT_pool", bufs=1)
            hT_pool = hT_ctx.__enter__()
            if True:
                hT = hT_pool.tile([P, NDT, T], bf16)
                hTq = hT_pool.tile([P, NDT, TQ], bf16)

                with tc.tile_pool(name="tpsum", bufs=8, space="PSUM") as tpsum:
                    def ln1_transpose(n_tiles, src_rows, dst):
                        for i in range(n_tiles):
                            x_i = hpool.tile([P, D], f32, tag="x_i", name=f"xi{i}")
                            nc.sync.dma_start(
                                out=x_i, in_=src_rows[i * P:(i + 1) * P, :]
                            )
                            _layer_norm_inplace(nc, stats, x_i, eps1)
                            for j0 in range(0, NDT, 4):
                                pt = tpsum.tile([P, 4, P], f32, tag="trans_f32")
                                for j in range(4):
                                    nc.tensor.transpose(
                                        pt[:, j, :],
                                        x_i[:, (j0 + j) * P:(j0 + j + 1) * P],
                                        ident_f,
                                    )
                                nc.scalar.copy(
                                    out=dst[:, j0:j0 + 4, i * P:(i + 1) * P], in_=pt
                                )

                    ln1_transpose(NKT, x_batch, hT)
                    ln1_transpose(NQT, x_own, hTq)

                # ---------- Stage B: K^T, V, Q^T ----------
                with tc.tile_pool(name="wstream", bufs=2) as wstream, \
                     tc.tile_pool(name="wvpool", bufs=8) as wvpool, \
                     tc.tile_pool(name="qkv_ps", bufs=8, space="PSUM") as qkv_ps:
                    # K^T[f, t] = sum_d Wk[d, f] * hT[d, t]   (+ bk)
                    for fp in range(NDT // 2):
                        wk_f = wstream.tile([P, NDT, 2 * P], bf16, tag="wk_f",
                                            name=f"wkf{fp}")
                        for d in range(NDT):
                            nc.sync.dma_start(
                                out=wk_f[:, d, :],
                                in_=Wk[d * P:(d + 1) * P,
                                       fp * 2 * P:(fp + 1) * 2 * P],
                            )
                        for half in range(2):
                            f = 2 * fp + half
                            pss = [qkv_ps.tile([P, 512], f32, tag="qkv",
                                               name=f"kps{f}_{n}")
                                   for n in range(4)]
                            for d in range(NDT):
                                for n in range(4):
                                    nc.tensor.matmul(
                                        pss[n], wk_f[:, d, half * P:(half + 1) * P],
                                        hT[:, d, n * 512:(n + 1) * 512],
                                        start=(d == 0), stop=(d == NDT - 1),
                                    )
                            for n in range(4):
                                nc.scalar.activation(
                                    out=KT[:, f, n * 512:(n + 1) * 512], in_=pss[n],
                                    func=AF.Identity, bias=bkT[:, f:f + 1],
                                    scale=1.0,
                                )
                    # Q^T scaled by 1/sqrt(HD)
                    for fp in range(NDT // 2):
                        wq_f = wstream.tile([P, NDT, 2 * P], bf16, tag="wk_f",
                                            name=f"wqf{fp}")
                        for d in range(NDT):
                            nc.sync.dma_start(
                                out=wq_f[:, d, :],
                                in_=Wq[d * P:(d + 1) * P,
                                       fp * 2 * P:(fp + 1) * 2 * P],
                            )
                        for half in range(2):
                            f = 2 * fp + half
                            ps = qkv_ps.tile([P, 512], f32, tag="qkv",
                                             name=f"qps{f}")
                            for d in range(NDT):
                                nc.tensor.matmul(
                                    ps, wq_f[:, d, half * P:(half + 1) * P],
                                    hTq[:, d, :],
                                    start=(d == 0), stop=(d == NDT - 1),
                                )
                            qb = stats.tile([P, 1], f32, tag="qb")
                            nc.vector.tensor_scalar_mul(qb, bqT[:, f:f + 1], 0.125)
                            nc.scalar.activation(
                                out=QT[:, f, :], in_=ps, func=AF.Identity,
                                bias=qb, scale=0.125,
                            )
                    # V[t, c] = sum_d h[t, d] Wv[d, c] + bv
                    for c in range(D // 512):
                        wv_d = []
                        for d in range(NDT):
                            wt = wvpool.tile([P, 512], bf16, tag="wv_d",
                                             name=f"wv_{c}_{d}")
                            nc.sync.dma_start(
                                out=wt,
                                in_=Wv[d * P:(d + 1) * P, c * 512:(c + 1) * 512],
                            )
                            wv_d.append(wt)
                        for tg in range(NKT // 4):
                            pss = [qkv_ps.tile([P, 512], f32, tag="qkv",
                                               name=f"vps{c}_{tg}_{i}")
                                   for i in range(4)]
                            for d in range(NDT):
                                for t4 in range(4):
                                    tt = tg * 4 + t4
                                    nc.tensor.matmul(
                                        pss[t4], hT[:, d, tt * P:(tt + 1) * P],
                                        wv_d[d], start=(d == 0), stop=False,
                                    )
                            for t4 in range(4):
                                tt = tg * 4 + t4
                                nc.tensor.matmul(
                                    pss[t4], ones_row,
                                    bv_row[0:1, c * 512:(c + 1) * 512],
                                    start=False, stop=True,
                                )
                                nc.scalar.copy(
                                    out=Vb[:, tt, c * 512:(c + 1) * 512],
                                    in_=pss[t4],
                                )

            hT_ctx.__exit__(None, None, None)
            # ---------- Stage C: attention, head pairs ----------
            with tc.tile_pool(name="maskp", bufs=4) as maskp, \
                 tc.tile_pool(name="spool", bufs=4) as spool, \
                 tc.tile_pool(name="abf_p", bufs=4) as abf_p, \
                 tc.tile_pool(name="etp", bufs=2) as etp, \
                 tc.tile_pool(name="rpool", bufs=3) as rpool, \
                 tc.tile_pool(name="s_ps", bufs=2, space="PSUM") as s_ps, \
                 tc.tile_pool(name="t_ps", bufs=2, space="PSUM") as t_ps, \
                 tc.tile_pool(name="pv_ps", bufs=1, space="PSUM") as pv_ps:
                for hp in range(H // 2):
                    fi = hp
                    rcol = [rpool.tile([P, NQT], f32, tag=f"rc{half}",
                                       name=f"rc{hp}_{half}") for half in range(2)]
                    ET = [etp.tile([P, NKT, TQ], bf16, tag="ET",
                                   name=f"ET{hp}_{half}") for half in range(2)]
                    for qt in range(NQT):
                        sms = []
                        mqs = []
                        for half in range(2):
                            h = 2 * hp + half
                            m_q = maskp.tile([P, T], u8, tag="m_q",
                                             name=f"m{h}_{qt}")
                            nc.sync.dma_start(
                                out=m_q, in_=mask_q[h, qt * P:(qt + 1) * P, :]
                            )
                            mqs.append(m_q)
                            sm = spool.tile([P, T], f32, tag="sm",
                                            name=f"sm{h}_{qt}")
                            sms.append(sm)
                        for kc2 in range(T // 1024):
                            pss_s = []
                            for half in range(2):
                                h = 2 * hp + half
                                r0 = half * 64
                                ps_s = s_ps.tile([P, 1024], f32, tag="s",
                                                 name=f"sps{h}_{qt}_{kc2}")
                                pss_s.append(ps_s)
                                for sub in range(2):
                                    kc = 2 * kc2 + sub
                                    nc.tensor.matmul(
                                        ps_s[:, sub * 512:(sub + 1) * 512],
                                        QT[r0:r0 + 64, fi, qt * P:(qt + 1) * P],
                                        KT[r0:r0 + 64, fi, kc * 512:(kc + 1) * 512],
                                        start=True, stop=True,
                                    )
                            for half in range(2):
                                nc.vector.scalar_tensor_tensor(
                                    out=sms[half][:, kc2 * 1024:(kc2 + 1) * 1024],
                                    in0=mqs[half][:, kc2 * 1024:(kc2 + 1) * 1024],
                                    scalar=float(NEG), in1=pss_s[half],
                                    op0=OP.mult, op1=OP.add,
                                )
                        for half in range(2):
                            h = 2 * hp + half
                            sm = sms[half]
                            sums = stats.tile([P, 1], f32, tag="sums")
                            # exp writes bf16 directly; PV uses it unnormalized
                            e_bf = abf_p.tile([P, T], bf16, tag="a_bf",
                                              name=f"abf{h}_{qt}")
                            nc.scalar.activation(
                                out=e_bf, in_=sm, func=AF.Exp, scale=1.0,
                                accum_out=sums,
                            )
                            rq = rcol[half][:, qt:qt + 1]
                            nc.vector.reciprocal(out=rq, in_=sums)
                            # normalized fp32 attn for HBM (bf16-rounded E * rq)
                            nc.vector.tensor_scalar_mul(sm, e_bf, rq)
                            nc.sync.dma_start(
                                out=attn_out[h, qt * P:(qt + 1) * P, :], in_=sm
                            )
                            for kj0 in range(0, NKT, 8):
                                pt = t_ps.tile([P, 8, P], bf16, tag="tbf")
                                for j in range(8):
                                    nc.tensor.transpose(
                                        pt[:, j, :],
                                        e_bf[:, (kj0 + j) * P:(kj0 + j + 1) * P],
                                        ident_bf,
                                    )
                                if kj0 == 0:
                                    nc.scalar.copy(
                                        out=ET[half][:, kj0:kj0 + 8,
                                                     qt * P:(qt + 1) * P],
                                        in_=pt,
                                    )
                                else:
                                    nc.vector.tensor_copy(
                                        out=ET[half][:, kj0:kj0 + 8,
                                                     qt * P:(qt + 1) * P],
                                        in_=pt,
                                    )
                    # PV for the pair (unnormalized), column-packed PSUM;
                    # normalize per query row during copyback.
                    ps_c = pv_ps.tile([P, TQ], f32, tag="pv", name=f"pv{hp}")
                    for kt in range(NKT):
                        nc.tensor.matmul(
                            ps_c[0:64], Vb[:, kt, (2 * hp) * HD:(2 * hp + 1) * HD],
                            ET[0][:, kt, :],
                            start=(kt == 0), stop=(kt == NKT - 1),
                            tile_position=(0, 0),
                        )
                        nc.tensor.matmul(
                            ps_c[64:128],
                            Vb[:, kt, (2 * hp + 1) * HD:(2 * hp + 2) * HD],
                            ET[1][:, kt, :],
                            start=(kt == 0), stop=(kt == NKT - 1),
                            tile_position=(0, 64),
                        )
                    R2 = rpool.tile([P, TQ], f32, tag="R2", name=f"R2_{hp}")
                    for half in range(2):
                        rt_ps = pv_ps.tile([NQT, P], f32, tag="rt",
                                          name=f"rt{hp}_{half}")
                        nc.tensor.transpose(rt_ps, rcol[half], ident_f)
                        rt_sb = rpool.tile([NQT, P], f32, tag="rt_sb",
                                           name=f"rtsb{hp}_{half}")
                        nc.scalar.copy(out=rt_sb, in_=rt_ps)
                        nc.sync.dma_start(out=r_dram[hp, half], in_=rt_sb)
                        for qt in range(NQT):
                            rsrc = r_dram[hp, half, qt]
                            src = bass.AP(
                                tensor=rsrc.tensor, offset=rsrc.offset,
                                ap=[[0, 64]] + rsrc.ap,
                            )
                            nc.sync.dma_start(
                                out=R2[half * 64:(half + 1) * 64,
                                       qt * P:(qt + 1) * P],
                                in_=src,
                            )
                    nc.vector.tensor_mul(
                        out=ctxT[:, fi, :], in0=ps_c, in1=R2
                    )
                    nc.sync.dma_start(out=ctx_dram[:, fi, :], in_=ctxT[:, fi, :])
            kv_ctx.__exit__(None, None, None)

            # ---------- Stage D: Wo + residual + LN2 ----------
            with tc.tile_pool(name="xpool2", bufs=1) as xpool2:
                x2 = xpool2.tile([P, NQT, D], f32)
                h2T = xpool2.tile([P, NDT, TQ], bf16)
                ctxT = xpool2.tile([P, NDT, TQ], bf16, name="ctxT2")
                for d in range(NDT):
                    nc.sync.dma_start(out=ctxT[:, d, :], in_=ctx_dram[:, d, :])
                with tc.tile_pool(name="dtmp", bufs=1) as dtmp:
                    bo_b = bcast(dtmp, bo, D, "bo_b")
                    xb = dtmp.tile([P, NQT, D], f32)
                    deltaT = dtmp.tile([P, NDT, TQ], f32)
                    wo_all = dtmp.tile([P, NDT, D], bf16)
                    for d in range(NDT):
                        nc.sync.dma_start(
                            out=wo_all[:, d, :], in_=Wo[d * P:(d + 1) * P, :]
                        )
                    for qt in range(NQT):
                        xo = hpool.tile([P, D], f32, tag="x_i", name=f"xo{qt}")
                        nc.sync.dma_start(out=xo, in_=x_own[qt * P:(qt + 1) * P, :])
                        nc.vector.tensor_add(out=xb[:, qt, :], in0=xo, in1=bo_b)
                    with tc.tile_pool(name="d_ps", bufs=8, space="PSUM") as d_ps:
                        pss_d = [d_ps.tile([P, TQ], f32, tag="dps",
                                           name=f"dps{f}") for f in range(NDT)]
                        for d in range(NDT):
                            for f in range(NDT):
                                nc.tensor.matmul(
                                    pss_d[f], wo_all[:, d, f * P:(f + 1) * P],
                                    ctxT[:, d, :],
                                    start=(d == 0), stop=(d == NDT - 1),
                                )
                        for f in range(NDT):
                            nc.scalar.copy(out=deltaT[:, f, :], in_=pss_d[f])
                    with tc.tile_pool(name="tpsum2", bufs=4, space="PSUM") as tpsum2:
                      for qt in range(NQT):
                        for f0 in range(0, NDT, 4):
                            pt = tpsum2.tile([P, 4, P], f32, tag="trans2")
                            for j in range(4):
                                nc.tensor.transpose(
                                    pt[:, j, :],
                                    deltaT[:, f0 + j, qt * P:(qt + 1) * P],
                                    ident_f,
                                )
                            nc.vector.tensor_add(
                                out=x2[:, qt, f0 * P:(f0 + 4) * P],
                                in0=pt.rearrange("p a b -> p (a b)"),
                                in1=xb[:, qt, f0 * P:(f0 + 4) * P],
                            )
                      for qt in range(NQT):
                        h2 = hpool.tile([P, D], f32, tag="x_i", name=f"h2{qt}")
                        nc.vector.tensor_copy(out=h2, in_=x2[:, qt, :])
                        _layer_norm_inplace(nc, stats, h2, eps1)
                        for j0 in range(0, NDT, 4):
                            pt = tpsum2.tile([P, 4, P], f32, tag="trans2")
                            for j in range(4):
                                nc.tensor.transpose(
                                    pt[:, j, :],
                                    h2[:, (j0 + j) * P:(j0 + j + 1) * P],
                                    ident_f,
                                )
                            nc.scalar.copy(
                                out=h2T[:, j0:j0 + 4, qt * P:(qt + 1) * P], in_=pt
                            )

                # ---------- Stage E: FFN ----------
                with tc.tile_pool(name="gpool", bufs=1) as gpool, \
                     tc.tile_pool(name="w1s", bufs=6) as w1s, \
                     tc.tile_pool(name="ff_ps", bufs=4, space="PSUM") as ff_ps:
                    gT = gpool.tile([P, NFT, TQ], bf16)
                    xout_sb = gpool.tile([P, NQT, D], f32)
                    for fp in range(NFT // 2):
                        w1_f = w1s.tile([P, NDT, 2 * P], bf16, tag="w1_f",
                                        name=f"w1f{fp}")
                        for d in range(NDT):
                            nc.sync.dma_start(
                                out=w1_f[:, d, :],
                                in_=W1[d * P:(d + 1) * P,
                                       fp * 2 * P:(fp + 1) * 2 * P],
                            )
                        for half in range(2):
                            ff = 2 * fp + half
                            ps = ff_ps.tile([P, TQ], f32, tag="ff1",
                                            name=f"ff1ps{ff}")
                            for d in range(NDT):
                                nc.tensor.matmul(
                                    ps, w1_f[:, d, half * P:(half + 1) * P],
                                    h2T[:, d, :],
                                    start=(d == 0), stop=(d == NDT - 1),
                                )
                            nc.scalar.activation(
                                out=gT[:, ff, :], in_=ps, func=AF.Gelu,
                                bias=b1T[:, ff:ff + 1], scale=1.0,
                            )
                    for c in range(D // 512):
                        pss = [ff_ps.tile([P, 512], f32, tag="ff2",
                                          name=f"ff2ps{c}_{i}") for i in range(NQT)]
                        for ff in range(NFT):
                            w2_f = w1s.tile([P, 512], bf16, tag="w2_f",
                                            name=f"w2f{c}_{ff}")
                            nc.sync.dma_start(
                                out=w2_f,
                                in_=W2[ff * P:(ff + 1) * P, c * 512:(c + 1) * 512],
                            )
                            for qt in range(NQT):
                                nc.tensor.matmul(
                                    pss[qt], gT[:, ff, qt * P:(qt + 1) * P],
                                    w2_f, start=(ff == 0), stop=False,
                                )
                        for qt in range(NQT):
                            nc.tensor.matmul(
                                pss[qt], ones_row,
                                b2_row[0:1, c * 512:(c + 1) * 512],
                                start=False, stop=True,
                            )
                            nc.vector.tensor_add(
                                out=xout_sb[:, qt, c * 512:(c + 1) * 512],
                                in0=pss[qt],
                                in1=x2[:, qt, c * 512:(c + 1) * 512],
                            )
                    for qt in range(NQT):
                        nc.sync.dma_start(
                            out=x_out[qt * P:(qt + 1) * P, :], in_=xout_sb[:, qt, :]
                        )
    nc.finalize()
    return nc


def _enable_ldw_opt():
    import os
    if os.environ.get("ENC_LDW_OPT") != "1":
        return
    import concourse.bass_utils as bu
    orig = bu.bir_verify_and_optimise
    if getattr(bu, "_ldw_patched", False):
        return
    def patched(*a, **kw):
        import subprocess
        orig_run = bu.run_command
        def run2(cmd, **k):
            cmd = ["--enable-ldw-opt=true" if c == "--enable-ldw-opt=false" else c
                   for c in cmd]
            return orig_run(cmd, **k)
        bu.run_command = run2
        try:
            return orig(*a, **kw)
        finally:
            bu.run_command = orig_run
    bu.bir_verify_and_optimise = patched
    bu._ldw_patched = True


def _get_nc():
    _enable_ldw_opt()
    if "nc" not in _CACHE:
        nc = bacc.Bacc("TRN2", target_bir_lowering=False, debug=False)
        build(nc)
        _CACHE["nc"] = nc
    return _CACHE["nc"]


def make_in_maps(inputs):
    from concourse.dt import dt as _dt

    np_bf16 = _dt.np(_dt.bfloat16)
    f = lambda k: np.asarray(inputs[k], np.float32)
    x = f("x")
    mask = np.asarray(inputs["mask"]).astype(np.uint8)
    # fold layer-norm affine params into the following projections (exact)
    lw1, lb1 = f("ln1_w"), f("ln1_b")
    lw2, lb2 = f("ln2_w"), f("ln2_b")
    common = {
        "bo": f("bo"), "b2": f("b2"),
        "b1": f("b1") + lb2 @ f("W1"),
        "bq": f("bq") + lb1 @ f("Wq"),
        "bk": f("bk") + lb1 @ f("Wk"),
        "bv": f("bv") + lb1 @ f("Wv"),
        "Wo": f("Wo").astype(np_bf16),
        "W2": f("W2").astype(np_bf16),
        "W1": (lw2[:, None] * f("W1")).astype(np_bf16),
    }
    for name in ("Wq", "Wk", "Wv"):
        common[name] = (lw1[:, None] * f(name)).astype(np_bf16)
    common = {k: np.ascontiguousarray(v) for k, v in common.items()}
    in_maps = []
    for c in range(8):
        b, s = c // 4, (c % 4) * TQ
        m = dict(common)
        m["x_batch"] = np.ascontiguousarray(x[b])
        m["x_own"] = np.ascontiguousarray(x[b, s:s + TQ])
        m["mask_q"] = np.ascontiguousarray(mask[b, :, s:s + TQ, :])
        in_maps.append(m)
    return in_maps


def run_cores(inputs, **kw):
    nc = _get_nc()
    return bass_utils.run_bass_kernel_spmd(
        nc, make_in_maps(inputs), core_ids=list(range(8)), **kw
    )


def kernel(**inputs):
    res = run_cores(inputs)
    x_full = np.empty((B, T, D), np.float32)
    attn_full = np.empty((B, H, T, T), np.float32)
    for c in range(8):
        b, s = c // 4, (c % 4) * TQ
        x_full[b, s:s + TQ, :] = res.results[c]["x_out"]
        attn_full[b, :, s:s + TQ, :] = res.results[c]["attn_out"]
    return x_full, attn_full


# revision 24
# speedup vs baseline: 1.0301x; 1.0301x over previous
"""Trainium2 Bass kernel for nn_EncoderLayer (B=2, T=2048, D=1024, H=16, DFF=4096).

Sharding: 8 cores; core c handles batch b=c//4, query rows q0=(c%4)*512..+512.
Each core redundantly computes LN1 + K,V for its full batch (no collectives),
then attention + output proj + FFN for its own 512 query rows.
Outputs per core: attn slice (16, 512, 2048) and x_out slice (512, 1024).

Numerics: layer-norm affine params are folded into the adjacent projection
weights on the host (exact). Projections/scores/PV/FFN matmuls run in bf16
with fp32 PSUM accumulation; softmax (mask, exp, normalize) runs in fp32.
"""

import sys

sys.path.insert(0, "/opt/trn_rl_repo")

import numpy as np

import concourse.bacc as bacc
import concourse.bass as bass
import concourse.tile as tile
from concourse import mybir
from concourse import bass_utils
from concourse.masks import make_identity

P = 128
B, T, D = 2, 2048, 1024
H, HD = 16, 64
DFF = 4096
TQ = 512          # query rows per core
NQT = TQ // P     # 4
NKT = T // P      # 16
NDT = D // P      # 8
NFT = DFF // P    # 32
EPS = 1e-5
NEG = -1e9        # exp(x + NEG) == 0.0 in fp32 for any realistic score x

f32 = mybir.dt.float32
f32r = mybir.dt.float32r
bf16 = mybir.dt.bfloat16
u8 = mybir.dt.uint8
AF = mybir.ActivationFunctionType
OP = mybir.AluOpType

_CACHE = {}


def _layer_norm_inplace(nc, stats, x_i, eps1):
    """x_i <- (x_i - mean) * rsqrt(var + eps) (ln affine folded into weights)."""
    st = stats.tile([P, 2, 6], f32, tag="bn")
    nc.vector.bn_stats(out=st[:, 0, :], in_=x_i[:, 0:512])
    nc.vector.bn_stats(out=st[:, 1, :], in_=x_i[:, 512:1024])
    mv = stats.tile([P, 2], f32, tag="mv")
    nc.vector.bn_aggr(out=mv, in_=st)
    mean = mv[:, 0:1]
    var = mv[:, 1:2]
    std = stats.tile([P, 1], f32, tag="std")
    nc.scalar.activation(out=std, in_=var, func=AF.Sqrt, bias=eps1, scale=1.0)
    rstd = stats.tile([P, 1], f32, tag="rstd")
    nc.vector.reciprocal(out=rstd, in_=std)
    negmur = stats.tile([P, 1], f32, tag="negmur")
    nc.vector.tensor_scalar(
        out=negmur, in0=mean, scalar1=rstd, scalar2=-1.0, op0=OP.mult, op1=OP.mult
    )
    nc.scalar.activation(out=x_i, in_=x_i, func=AF.Identity, bias=negmur, scale=rstd)


def build(nc):
    # ---------------- DRAM I/O ----------------
    x_batch = nc.dram_tensor("x_batch", (T, D), f32, kind="ExternalInput").ap()
    x_own = nc.dram_tensor("x_own", (TQ, D), f32, kind="ExternalInput").ap()
    mask_q = nc.dram_tensor("mask_q", (H, TQ, T), u8, kind="ExternalInput").ap()
    Wq = nc.dram_tensor("Wq", (D, D), bf16, kind="ExternalInput").ap()
    Wk = nc.dram_tensor("Wk", (D, D), bf16, kind="ExternalInput").ap()
    Wv = nc.dram_tensor("Wv", (D, D), bf16, kind="ExternalInput").ap()
    Wo = nc.dram_tensor("Wo", (D, D), bf16, kind="ExternalInput").ap()
    W1 = nc.dram_tensor("W1", (D, DFF), bf16, kind="ExternalInput").ap()
    W2 = nc.dram_tensor("W2", (DFF, D), bf16, kind="ExternalInput").ap()
    bq = nc.dram_tensor("bq", (D,), f32, kind="ExternalInput").ap()
    bk = nc.dram_tensor("bk", (D,), f32, kind="ExternalInput").ap()
    bv = nc.dram_tensor("bv", (D,), f32r, kind="ExternalInput").ap()
    bo = nc.dram_tensor("bo", (D,), f32, kind="ExternalInput").ap()
    b1 = nc.dram_tensor("b1", (DFF,), f32, kind="ExternalInput").ap()
    b2 = nc.dram_tensor("b2", (D,), f32r, kind="ExternalInput").ap()
    attn_out = nc.dram_tensor("attn_out", (H, TQ, T), f32, kind="ExternalOutput").ap()
    x_out = nc.dram_tensor("x_out", (TQ, D), f32, kind="ExternalOutput").ap()
    ctx_dram = nc.dram_tensor("ctx_scratch", (P, NDT, TQ), bf16).ap()
    r_dram = nc.dram_tensor("r_scratch", (H // 2, 2, NQT, P), f32).ap()

    def bcast(pool, src, n, tag):
        t = pool.tile([P, n], f32, tag=tag, name=tag)
        src_b = bass.AP(tensor=src.tensor, offset=src.offset, ap=[[0, P]] + src.ap)
        nc.sync.dma_start(out=t, in_=src_b)
        return t

    with tile.TileContext(nc) as tc:
        with tc.tile_pool(name="consts", bufs=1) as consts, \
             tc.tile_pool(name="stats", bufs=8) as stats, \
             tc.tile_pool(name="hpool", bufs=5) as hpool:
            ident_bf = consts.tile([P, P], bf16)
            make_identity(nc, ident_bf)
            ident_f = consts.tile([P, P], f32)
            make_identity(nc, ident_f)
            eps1 = consts.tile([P, 1], f32)
            nc.vector.memset(eps1, EPS)
            ones_f = consts.tile([1, P], f32)
            nc.vector.memset(ones_f, 1.0)
            ones_row = consts.tile([1, P], f32r)
            nc.scalar.copy(out=ones_row, in_=ones_f)
            bkT = consts.tile([P, NDT], f32)
            nc.sync.dma_start(out=bkT, in_=bk.rearrange("(f p) -> p f", p=P))
            bqT = consts.tile([P, NDT], f32)
            nc.sync.dma_start(out=bqT, in_=bq.rearrange("(f p) -> p f", p=P))
            b1T = consts.tile([P, NFT], f32)
            nc.sync.dma_start(out=b1T, in_=b1.rearrange("(f p) -> p f", p=P))
            bv_row = consts.tile([1, D], f32r)
            nc.sync.dma_start(out=bv_row, in_=bv[None, :])
            b2_row = consts.tile([1, D], f32r)
            nc.sync.dma_start(out=b2_row, in_=b2[None, :])

            # ---------- Stages A-C under kv_pool ----------
            kv_ctx = tc.tile_pool(name="kv_pool", bufs=1)
            kv_pool = kv_ctx.__enter__()
            ctxT = kv_pool.tile([P, NDT, TQ], bf16)   # ctx^T [feat, qtok]  8KB/part
            KT = kv_pool.tile([P, NDT, T], bf16)      # K^T  [feat, tok]   32KB/part
            Vb = kv_pool.tile([P, NKT, D], bf16)      # V    [tok, feat]   32KB/part
            QT = kv_pool.tile([P, NDT, TQ], bf16)     # Q^T/8 [feat, qtok]  8KB/part

            # ---------- Stage A: LN1 -> hT ----------
            hT_ctx = tc.tile_pool(name="hT_pool", bufs=1)
            hT_pool = hT_ctx.__enter__()
            if True:
                hT = hT_pool.tile([P, NDT, T], bf16)
                hTq = hT_pool.tile([P, NDT, TQ], bf16)

                with tc.tile_pool(name="tpsum", bufs=8, space="PSUM") as tpsum:
                    def ln1_transpose(n_tiles, src_rows, dst):
                        for i in range(n_tiles):
                            x_i = hpool.tile([P, D], f32, tag="x_i", name=f"xi{i}")
                            nc.sync.dma_start(
                                out=x_i, in_=src_rows[i * P:(i + 1) * P, :]
                            )
                            _layer_norm_inplace(nc, stats, x_i, eps1)
                            for j0 in range(0, NDT, 4):
                                pt = tpsum.tile([P, 4, P], f32, tag="trans_f32")
                                for j in range(4):
                                    nc.tensor.transpose(
                                        pt[:, j, :],
                                        x_i[:, (j0 + j) * P:(j0 + j + 1) * P],
                                        ident_f,
                                    )
                                nc.scalar.copy(
                                    out=dst[:, j0:j0 + 4, i * P:(i + 1) * P], in_=pt
                                )

                    ln1_transpose(NKT, x_batch, hT)
                    ln1_transpose(NQT, x_own, hTq)

                # ---------- Stage B: K^T, V, Q^T ----------
                with tc.tile_pool(name="wstream", bufs=2) as wstream, \
                     tc.tile_pool(name="wvpool", bufs=8) as wvpool, \
                     tc.tile_pool(name="qkv_ps", bufs=8, space="PSUM") as qkv_ps:
                    # K^T[f, t] = sum_d Wk[d, f] * hT[d, t]   (+ bk)
                    for fp in range(NDT // 2):
                        wk_f = wstream.tile([P, NDT, 2 * P], bf16, tag="wk_f",
                                            name=f"wkf{fp}")
                        for d in range(NDT):
                            nc.sync.dma_start(
                                out=wk_f[:, d, :],
                                in_=Wk[d * P:(d + 1) * P,
                                       fp * 2 * P:(fp + 1) * 2 * P],
                            )
                        for half in range(2):
                            f = 2 * fp + half
                            pss = [qkv_ps.tile([P, 512], f32, tag="qkv",
                                               name=f"kps{f}_{n}")
                                   for n in range(4)]
                            for d in range(NDT):
                                for n in range(4):
                                    nc.tensor.matmul(
                                        pss[n], wk_f[:, d, half * P:(half + 1) * P],
                                        hT[:, d, n * 512:(n + 1) * 512],
                                        start=(d == 0), stop=(d == NDT - 1),
                                    )
                            for n in range(4):
                                nc.scalar.activation(
                                    out=KT[:, f, n * 512:(n + 1) * 512], in_=pss[n],
                                    func=AF.Identity, bias=bkT[:, f:f + 1],
                                    scale=1.0,
                                )
                    # Q^T scaled by 1/sqrt(HD)
                    for fp in range(NDT // 2):
                        wq_f = wstream.tile([P, NDT, 2 * P], bf16, tag="wk_f",
                                            name=f"wqf{fp}")
                        for d in range(NDT):
                            nc.sync.dma_start(
                                out=wq_f[:, d, :],
                                in_=Wq[d * P:(d + 1) * P,
                                       fp * 2 * P:(fp + 1) * 2 * P],
                            )
                        for half in range(2):
                            f = 2 * fp + half
                            ps = qkv_ps.tile([P, 512], f32, tag="qkv",
                                             name=f"qps{f}")
                            for d in range(NDT):
                                nc.tensor.matmul(
                                    ps, wq_f[:, d, half * P:(half + 1) * P],
                                    hTq[:, d, :],
                                    start=(d == 0), stop=(d == NDT - 1),
                                )
                            qb = stats.tile([P, 1], f32, tag="qb")
                            nc.vector.tensor_scalar_mul(qb, bqT[:, f:f + 1], 0.125)
                            nc.scalar.activation(
                                out=QT[:, f, :], in_=ps, func=AF.Identity,
                                bias=qb, scale=0.125,
                            )
                    # V[t, c] = sum_d h[t, d] Wv[d, c] + bv
                    for c in range(D // 512):
                        wv_d = []
                        for d in range(NDT):
                            wt = wvpool.tile([P, 512], bf16, tag="wv_d",
                                             name=f"wv_{c}_{d}")
                            nc.sync.dma_start(
                                out=wt,
                                in_=Wv[d * P:(d + 1) * P, c * 512:(c + 1) * 512],
                            )
                            wv_d.append(wt)
                        for tg in range(NKT // 4):
                            pss = [qkv_ps.tile([P, 512], f32, tag="qkv",
                                               name=f"vps{c}_{tg}_{i}")
                                   for i in range(4)]
                            for d in range(NDT):
                                for t4 in range(4):
                                    tt = tg * 4 + t4
                                    nc.tensor.matmul(
                                        pss[t4], hT[:, d, tt * P:(tt + 1) * P],
                                        wv_d[d], start=(d == 0), stop=False,
                                    )
                            for t4 in range(4):
                                tt = tg * 4 + t4
                                nc.tensor.matmul(
                                    pss[t4], ones_row,
                                    bv_row[0:1, c * 512:(c + 1) * 512],
                                    start=False, stop=True,
                                )
                                nc.scalar.copy(
                                    out=Vb[:, tt, c * 512:(c + 1) * 512],
                                    in_=pss[t4],
                                )

            hT_ctx.__exit__(None, None, None)
            # ---------- Stage C: attention, head pairs ----------
            with tc.tile_pool(name="maskp", bufs=4) as maskp, \
                 tc.tile_pool(name="spool", bufs=4) as spool, \
                 tc.tile_pool(name="abf_p", bufs=4) as abf_p, \
                 tc.tile_pool(name="etp", bufs=2) as etp, \
                 tc.tile_pool(name="rpool", bufs=3) as rpool, \
                 tc.tile_pool(name="s_ps", bufs=2, space="PSUM") as s_ps, \
                 tc.tile_pool(name="t_ps", bufs=2, space="PSUM") as t_ps, \
                 tc.tile_pool(name="pv_ps", bufs=1, space="PSUM") as pv_ps:
                for hp in range(H // 2):
                    fi = hp
                    rcol = [rpool.tile([P, NQT], f32, tag=f"rc{half}",
                                       name=f"rc{hp}_{half}") for half in range(2)]
                    ET = [etp.tile([P, NKT, TQ], bf16, tag="ET",
                                   name=f"ET{hp}_{half}") for half in range(2)]
                    for qt in range(NQT):
                        sms = []
                        mqs = []
                        for half in range(2):
                            h = 2 * hp + half
                            m_q = maskp.tile([P, T], u8, tag="m_q",
                                             name=f"m{h}_{qt}")
                            nc.sync.dma_start(
                                out=m_q, in_=mask_q[h, qt * P:(qt + 1) * P, :]
                            )
                            mqs.append(m_q)
                            sm = spool.tile([P, T], f32, tag="sm",
                                            name=f"sm{h}_{qt}")
                            sms.append(sm)
                        for kc2 in range(T // 1024):
                            pss_s = []
                            for half in range(2):
                                h = 2 * hp + half
                                r0 = half * 64
                                ps_s = s_ps.tile([P, 1024], f32, tag="s",
                                                 name=f"sps{h}_{qt}_{kc2}")
                                pss_s.append(ps_s)
                                for sub in range(2):
                                    kc = 2 * kc2 + sub
                                    nc.tensor.matmul(
                                        ps_s[:, sub * 512:(sub + 1) * 512],
                                        QT[r0:r0 + 64, fi, qt * P:(qt + 1) * P],
                                        KT[r0:r0 + 64, fi, kc * 512:(kc + 1) * 512],
                                        start=True, stop=True,
                                    )
                            for half in range(2):
                                nc.vector.scalar_tensor_tensor(
                                    out=sms[half][:, kc2 * 1024:(kc2 + 1) * 1024],
                                    in0=mqs[half][:, kc2 * 1024:(kc2 + 1) * 1024],
                                    scalar=float(NEG), in1=pss_s[half],
                                    op0=OP.mult, op1=OP.add,
                                )
                        for half in range(2):
                            h = 2 * hp + half
                            sm = sms[half]
                            sums = stats.tile([P, 1], f32, tag="sums")
                            # exp writes bf16 directly; PV uses it unnormalized
                            e_bf = abf_p.tile([P, T], bf16, tag="a_bf",
                                              name=f"abf{h}_{qt}")
                            nc.scalar.activation(
                                out=e_bf, in_=sm, func=AF.Exp, scale=1.0,
                                accum_out=sums,
                            )
                            rq = rcol[half][:, qt:qt + 1]
                            nc.vector.reciprocal(out=rq, in_=sums)
                            # normalized fp32 attn for HBM (bf16-rounded E * rq)
                            nc.vector.tensor_scalar_mul(sm, e_bf, rq)
                            nc.sync.dma_start(
                                out=attn_out[h, qt * P:(qt + 1) * P, :], in_=sm
                            )
                            for kj0 in range(0, NKT, 8):
                                pt = t_ps.tile([P, 8, P], bf16, tag="tbf")
                                for j in range(8):
                                    nc.tensor.transpose(
                                        pt[:, j, :],
                                        e_bf[:, (kj0 + j) * P:(kj0 + j + 1) * P],
                                        ident_bf,
                                    )
                                nc.scalar.copy(
                                    out=ET[half][:, kj0:kj0 + 8,
                                                 qt * P:(qt + 1) * P],
                                    in_=pt,
                                )
                    # PV for the pair (unnormalized), column-packed PSUM;
                    # normalize per query row during copyback.
                    ps_c = pv_ps.tile([P, TQ], f32, tag="pv", name=f"pv{hp}")
                    for kt in range(NKT):
                        nc.tensor.matmul(
                            ps_c[0:64], Vb[:, kt, (2 * hp) * HD:(2 * hp + 1) * HD],
                            ET[0][:, kt, :],
                            start=(kt == 0), stop=(kt == NKT - 1),
                            tile_position=(0, 0),
                        )
                        nc.tensor.matmul(
                            ps_c[64:128],
                            Vb[:, kt, (2 * hp + 1) * HD:(2 * hp + 2) * HD],
                            ET[1][:, kt, :],
                            start=(kt == 0), stop=(kt == NKT - 1),
                            tile_position=(0, 64),
                        )
                    R2 = rpool.tile([P, TQ], f32, tag="R2", name=f"R2_{hp}")
                    for half in range(2):
                        rt_ps = pv_ps.tile([NQT, P], f32, tag="rt",
                                          name=f"rt{hp}_{half}")
                        nc.tensor.transpose(rt_ps, rcol[half], ident_f)
                        rt_sb = rpool.tile([NQT, P], f32, tag="rt_sb",
                                           name=f"rtsb{hp}_{half}")
                        nc.scalar.copy(out=rt_sb, in_=rt_ps)
                        nc.sync.dma_start(out=r_dram[hp, half], in_=rt_sb)
                        for qt in range(NQT):
                            rsrc = r_dram[hp, half, qt]
                            src = bass.AP(
                                tensor=rsrc.tensor, offset=rsrc.offset,
                                ap=[[0, 64]] + rsrc.ap,
                            )
                            nc.sync.dma_start(
                                out=R2[half * 64:(half + 1) * 64,
                                       qt * P:(qt + 1) * P],
                                in_=src,
                            )
                    nc.vector.tensor_mul(
                        out=ctxT[:, fi, :], in0=ps_c, in1=R2
                    )
                    nc.sync.dma_start(out=ctx_dram[:, fi, :], in_=ctxT[:, fi, :])
            kv_ctx.__exit__(None, None, None)

            # ---------- Stage D: Wo + residual + LN2 ----------
            with tc.tile_pool(name="xpool2", bufs=1) as xpool2:
                x2 = xpool2.tile([P, NQT, D], f32)
                h2T = xpool2.tile([P, NDT, TQ], bf16)
                ctxT = xpool2.tile([P, NDT, TQ], bf16, name="ctxT2")
                for d in range(NDT):
                    nc.sync.dma_start(out=ctxT[:, d, :], in_=ctx_dram[:, d, :])
                with tc.tile_pool(name="dtmp", bufs=1) as dtmp:
                    bo_b = bcast(dtmp, bo, D, "bo_b")
                    xb = dtmp.tile([P, NQT, D], f32)
                    deltaT = dtmp.tile([P, NDT, TQ], f32)
                    wo_all = dtmp.tile([P, NDT, D], bf16)
                    for d in range(NDT):
                        nc.sync.dma_start(
                            out=wo_all[:, d, :], in_=Wo[d * P:(d + 1) * P, :]
                        )
                    for qt in range(NQT):
                        xo = hpool.tile([P, D], f32, tag="x_i", name=f"xo{qt}")
                        nc.sync.dma_start(out=xo, in_=x_own[qt * P:(qt + 1) * P, :])
                        nc.vector.tensor_add(out=xb[:, qt, :], in0=xo, in1=bo_b)
                    with tc.tile_pool(name="d_ps", bufs=8, space="PSUM") as d_ps:
                        pss_d = [d_ps.tile([P, TQ], f32, tag="dps",
                                           name=f"dps{f}") for f in range(NDT)]
                        for d in range(NDT):
                            for f in range(NDT):
                                nc.tensor.matmul(
                                    pss_d[f], wo_all[:, d, f * P:(f + 1) * P],
                                    ctxT[:, d, :],
                                    start=(d == 0), stop=(d == NDT - 1),
                                )
                        for f in range(NDT):
                            nc.scalar.copy(out=deltaT[:, f, :], in_=pss_d[f])
                    with tc.tile_pool(name="tpsum2", bufs=4, space="PSUM") as tpsum2:
                      for qt in range(NQT):
                        for f0 in range(0, NDT, 4):
                            pt = tpsum2.tile([P, 4, P], f32, tag="trans2")
                            for j in range(4):
                                nc.tensor.transpose(
                                    pt[:, j, :],
                                    deltaT[:, f0 + j, qt * P:(qt + 1) * P],
                                    ident_f,
                                )
                            nc.vector.tensor_add(
                                out=x2[:, qt, f0 * P:(f0 + 4) * P],
                                in0=pt.rearrange("p a b -> p (a b)"),
                                in1=xb[:, qt, f0 * P:(f0 + 4) * P],
                            )
                      for qt in range(NQT):
                        h2 = hpool.tile([P, D], f32, tag="x_i", name=f"h2{qt}")
                        nc.vector.tensor_copy(out=h2, in_=x2[:, qt, :])
                        _layer_norm_inplace(nc, stats, h2, eps1)
                        for j0 in range(0, NDT, 4):
                            pt = tpsum2.tile([P, 4, P], f32, tag="trans2")
                            for j in range(4):
                                nc.tensor.transpose(
                                    pt[:, j, :],
                                    h2[:, (j0 + j) * P:(j0 + j + 1) * P],
                                    ident_f,
                                )
                            nc.scalar.copy(
                                out=h2T[:, j0:j0 + 4, qt * P:(qt + 1) * P], in_=pt
                            )

                # ---------- Stage E: FFN ----------
                with tc.tile_pool(name="gpool", bufs=1) as gpool, \
                     tc.tile_pool(name="w1s", bufs=6) as w1s, \
                     tc.tile_pool(name="ff_ps", bufs=4, space="PSUM") as ff_ps:
                    gT = gpool.tile([P, NFT, TQ], bf16)
                    xout_sb = gpool.tile([P, NQT, D], f32)
                    for fp in range(NFT // 2):
                        w1_f = w1s.tile([P, NDT, 2 * P], bf16, tag="w1_f",
                                        name=f"w1f{fp}")
                        for d in range(NDT):
                            nc.sync.dma_start(
                                out=w1_f[:, d, :],
                                in_=W1[d * P:(d + 1) * P,
                                       fp * 2 * P:(fp + 1) * 2 * P],
                            )
                        for half in range(2):
                            ff = 2 * fp + half
                            ps = ff_ps.tile([P, TQ], f32, tag="ff1",
                                            name=f"ff1ps{ff}")
                            for d in range(NDT):
                                nc.tensor.matmul(
                                    ps, w1_f[:, d, half * P:(half + 1) * P],
                                    h2T[:, d, :],
                                    start=(d == 0), stop=(d == NDT - 1),
                                )
                            nc.scalar.activation(
                                out=gT[:, ff, :], in_=ps, func=AF.Gelu,
                                bias=b1T[:, ff:ff + 1], scale=1.0,
                            )
                    for c in range(D // 512):
                        pss = [ff_ps.tile([P, 512], f32, tag="ff2",
                                          name=f"ff2ps{c}_{i}") for i in range(NQT)]
                        for ff in range(NFT):
                            w2_f = w1s.tile([P, 512], bf16, tag="w2_f",
                                            name=f"w2f{c}_{ff}")
                            nc.sync.dma_start(
                                out=w2_f,
                                in_=W2[ff * P:(ff + 1) * P, c * 512:(c + 1) * 512],
                            )
                            for qt in range(NQT):
                                nc.tensor.matmul(
                                    pss[qt], gT[:, ff, qt * P:(qt + 1) * P],
                                    w2_f, start=(ff == 0), stop=False,
                                )
                        for qt in range(NQT):
                            nc.tensor.matmul(
                                pss[qt], ones_row,
                                b2_row[0:1, c * 512:(c + 1) * 512],
                                start=False, stop=True,
                            )
                            nc.vector.tensor_add(
                                out=xout_sb[:, qt, c * 512:(c + 1) * 512],
                                in0=pss[qt],
                                in1=x2[:, qt, c * 512:(c + 1) * 512],
                            )
                    for qt in range(NQT):
                        nc.sync.dma_start(
                            out=x_out[qt * P:(qt + 1) * P, :], in_=xout_sb[:, qt, :]
                        )
    nc.finalize()
    return nc


def _enable_ldw_opt():
    import os
    if os.environ.get("ENC_LDW_OPT") != "1":
        return
    import concourse.bass_utils as bu
    orig = bu.bir_verify_and_optimise
    if getattr(bu, "_ldw_patched", False):
        return
    def patched(*a, **kw):
        import subprocess
        orig_run = bu.run_command
        def run2(cmd, **k):
            cmd = ["--enable-ldw-opt=true" if c == "--enable-ldw-opt=false" else c
                   for c in cmd]
            return orig_run(cmd, **k)
        bu.run_command = run2
        try:
            return orig(*a, **kw)
        finally:
            bu.run_command = orig_run
    bu.bir_verify_and_optimise = patched
    bu._ldw_patched = True


def _get_nc():
    _enable_ldw_opt()
    if "nc" not in _CACHE:
        nc = bacc.Bacc("TRN2", target_bir_lowering=False, debug=False)
        build(nc)
        _CACHE["nc"] = nc
    return _CACHE["nc"]


def make_in_maps(inputs):
    from concourse.dt import dt as _dt

    np_bf16 = _dt.np(_dt.bfloat16)
    f = lambda k: np.asarray(inputs[k], np.float32)
    x = f("x")
    mask = np.asarray(inputs["mask"]).astype(np.uint8)
    # fold layer-norm affine params into the following projections (exact)
    lw1, lb1 = f("ln1_w"), f("ln1_b")
    lw2, lb2 = f("ln2_w"), f("ln2_b")
    common = {
        "bo": f("bo"), "b2": f("b2"),
        "b1": f("b1") + lb2 @ f("W1"),
        "bq": f("bq") + lb1 @ f("Wq"),
        "bk": f("bk") + lb1 @ f("Wk"),
        "bv": f("bv") + lb1 @ f("Wv"),
        "Wo": f("Wo").astype(np_bf16),
        "W2": f("W2").astype(np_bf16),
        "W1": (lw2[:, None] * f("W1")).astype(np_bf16),
    }
    for name in ("Wq", "Wk", "Wv"):
        common[name] = (lw1[:, None] * f(name)).astype(np_bf16)
    common = {k: np.ascontiguousarray(v) for k, v in common.items()}
    in_maps = []
    for c in range(8):
        b, s = c // 4, (c % 4) * TQ
        m = dict(common)
        m["x_batch"] = np.ascontiguousarray(x[b])
        m["x_own"] = np.ascontiguousarray(x[b, s:s + TQ])
        m["mask_q"] = np.ascontiguousarray(mask[b, :, s:s + TQ, :])
        in_maps.append(m)
    return in_maps


def run_cores(inputs, **kw):
    nc = _get_nc()
    return bass_utils.run_bass_kernel_spmd(
        nc, make_in_maps(inputs), core_ids=list(range(8)), **kw
    )


def kernel(**inputs):
    res = run_cores(inputs)
    x_full = np.empty((B, T, D), np.float32)
    attn_full = np.empty((B, H, T, T), np.float32)
    for c in range(8):
        b, s = c // 4, (c % 4) * TQ
        x_full[b, s:s + TQ, :] = res.results[c]["x_out"]
        attn_full[b, :, s:s + TQ, :] = res.results[c]["attn_out"]
    return x_full, attn_full
